# revision 2
# baseline (speedup 1.0000x reference)
"""Trainium2 Bass kernel: batched multi-head attention (B=2, H=16, S=2048, D=64).

out = 2.0 * softmax(0.2 * Q @ K^T) @ V, computed per (b, h) pair.

Sharding: the 32 (b,h) pairs are split across 8 NeuronCores, 4 heads per core.
Each core runs an identical Bass program on its own slice; no communication.

Per-head dataflow on a core (scores kept TRANSPOSED so exp is the only
elementwise pass over the S x S matrix):
  - Paired PE transposes build K^T with k-tile pairs stacked on partitions
    0-63 / 64-127, and Q^T duplicated on both partition halves.
  - QK^T: two concurrent K=64 matmuls on disjoint row groups write
    scores^T [128k x 2x512q] tiles into PSUM.
  - ACT exp(0.2*x) reads PSUM, writes P^T to SBUF (layout already correct
    for the AV matmul).
  - AV: lhsT = [V | 0.5] (65 cols), accumulating out^T[65, 512] over all 16
    k-tiles; row 64 = 0.5 * sum(exp) so its reciprocal is 2/denominator.
  - PE transposes out^T back in 128-col chunks; DVE applies the reciprocal;
    DMA out. Max subtraction is skipped: |0.2*scores| <= ~10, safe in fp32.
"""

import numpy as np

B, H, S, D = 2, 16, 2048, 64
N_CORES = 8
HPC = (B * H) // N_CORES  # heads per core = 4
SCALE = 0.2
FACTOR = 2.0

P = 128  # s-tile (partition) size
QB = 512  # q block = one PSUM bank of fp32

_PROGRAM_CACHE = {}


def _build_program(hpc=HPC, s=S):
    from contextlib import ExitStack

    import concourse.bass as bass  # noqa: F401
    import concourse.mybir as mybir
    import concourse.tile as tile
    from concourse import bacc
    from concourse.masks import make_identity

    f32 = mybir.dt.float32
    Exp = mybir.ActivationFunctionType.Exp

    nt = s // P  # s-tiles per head
    npair = nt // 2  # k-tile pairs
    nqb = s // QB  # q blocks per head

    nc = bacc.Bacc("TRN2", target_bir_lowering=False, debug=False)

    q_d = nc.dram_tensor("q", [hpc, s, D], f32, kind="ExternalInput").ap()
    k_d = nc.dram_tensor("k", [hpc, s, D], f32, kind="ExternalInput").ap()
    v_d = nc.dram_tensor("v", [hpc, s, D], f32, kind="ExternalInput").ap()
    o_d = nc.dram_tensor("o", [hpc, s, D], f32, kind="ExternalOutput").ap()

    with tile.TileContext(nc) as tc:
        with ExitStack() as ctx:
            const = ctx.enter_context(tc.tile_pool(name="const", bufs=1))
            pin = ctx.enter_context(tc.tile_pool(name="pin", bufs=6))
            kq = ctx.enter_context(tc.tile_pool(name="kq", bufs=2))
            vpool = ctx.enter_context(tc.tile_pool(name="vpool", bufs=2 * nt))
            ptp = ctx.enter_context(tc.tile_pool(name="ptp", bufs=8))
            otp = ctx.enter_context(tc.tile_pool(name="otp", bufs=3))
            osb = ctx.enter_context(tc.tile_pool(name="osb", bufs=6))
            # PSUM: "ps" = 2 slots x 2 banks (scores + transpose staging),
            # "acc" = 4 slots x 1 bank (out^T accumulators).
            ps = ctx.enter_context(tc.tile_pool(name="ps", bufs=2, space="PSUM"))
            acc = ctx.enter_context(tc.tile_pool(name="acc", bufs=4, space="PSUM"))

            ident = const.tile([P, P], f32)
            make_identity(nc, ident)

            # Warm the exp table set early so the ~2.7us load overlaps DMAs.
            warm = const.tile([P, 8], f32)
            nc.vector.memset(warm, 0.0)
            nc.scalar.activation(out=warm, in_=warm, func=Exp, scale=1.0)

            # Per-head persistent state (keyed by head index).
            kT = {}  # [128, npair, 128]; rows 0:63 = K_{2p}^T, 64:128 = K_{2p+1}^T
            qT = {}  # [128, s]; rows 0:63 = Q^T, rows 64:128 = duplicate
            vt = {}  # list of [128, 65] tiles: [V_t | 0.5]

            def load_k_pair(h, p):
                def op():
                    t = pin.tile([P, P], f32, name=f"kin_{h}_{p}", tag="pin")
                    nc.sync.dma_start(out=t[:, 0:D], in_=k_d[h, (2 * p) * P : (2 * p + 1) * P, :])
                    nc.sync.dma_start(out=t[:, D : 2 * D], in_=k_d[h, (2 * p + 1) * P : (2 * p + 2) * P, :])
                    tp = ps.tile([P, P], f32, name=f"ktp_{h}_{p}", tag="sc")
                    nc.tensor.transpose(tp, t, ident)
                    nc.vector.tensor_copy(out=kT[h][:, p, :], in_=tp)

                return op

            def load_q_tile(h, t_idx):
                def op():
                    t = pin.tile([P, P], f32, name=f"qin_{h}_{t_idx}", tag="pin")
                    src = q_d[h, t_idx * P : (t_idx + 1) * P, :]
                    nc.sync.dma_start(out=t[:, 0:D], in_=src)
                    nc.sync.dma_start(out=t[:, D : 2 * D], in_=src)
                    tp = ps.tile([P, P], f32, name=f"qtp_{h}_{t_idx}", tag="sc")
                    nc.tensor.transpose(tp, t, ident)
                    nc.vector.tensor_copy(out=qT[h][:, t_idx * P : (t_idx + 1) * P], in_=tp)

                return op

            def load_v_tile(h, t_idx):
                def op():
                    t = vpool.tile([P, D + 1], f32, name=f"v_{h}_{t_idx}", tag="v")
                    nc.sync.dma_start(out=t[:, 0:D], in_=v_d[h, t_idx * P : (t_idx + 1) * P, :])
                    nc.gpsimd.memset(t[:, D : D + 1], 0.5)
                    vt[h].append(t)

                return op

            def input_stage(h):
                kT[h] = kq.tile([P, npair, P], f32, name=f"kT_{h}", tag="kT")
                qT[h] = kq.tile([P, s], f32, name=f"qT_{h}", tag="qT")
                vt[h] = []
                ops = []
                for p in range(npair):
                    ops.append(load_k_pair(h, p))
                for t_idx in range(nt):
                    ops.append(load_q_tile(h, t_idx))
                for t_idx in range(nt):
                    ops.append(load_v_tile(h, t_idx))
                return ops

            def out_chunk(h, qb, oT, c):
                def op():
                    tp = ps.tile([P, D + 1], f32, name=f"otr_{h}_{qb}_{c}", tag="sc")
                    nc.tensor.transpose(tp, oT[:, c * P : (c + 1) * P], ident[0 : D + 1, 0 : D + 1])
                    recip = osb.tile([P, 1], f32, name=f"rcp_{h}_{qb}_{c}", tag="rcp")
                    nc.vector.reciprocal(out=recip, in_=tp[:, D : D + 1])
                    ob = osb.tile([P, D], f32, name=f"ob_{h}_{qb}_{c}", tag="ob")
                    nc.vector.tensor_scalar_mul(out=ob, in0=tp[:, 0:D], scalar1=recip)
                    nc.sync.dma_start(out=o_d[h, qb * QB + c * P : qb * QB + (c + 1) * P, :], in_=ob)

                return op

            def output_stage(h, out_tiles):
                ops = []
                oTs = {}

                def evac(qb):
                    def op():
                        oT = otp.tile([D + 1, QB], f32, name=f"oT_{h}_{qb}", tag="oT")
                        nc.vector.tensor_copy(out=oT, in_=out_tiles[qb])
                        oTs[qb] = oT

                    return op

                for qb in range(nqb):
                    ops.append(evac(qb))
                for qb in range(nqb):
                    for c in range(QB // P):
                        # oTs[qb] resolved lazily (evac runs first in emission order)
                        ops.append(lambda qb=qb, c=c: out_chunk(h, qb, oTs[qb], c)())
                return ops

            def emit_qk_exp(h, p, pT_store):
                # QK^T + exp for pair p over all q-blocks, two q-blocks at a
                # time so the 2 score slots never deadlock and LDWEIGHTS
                # switches only 2x per group of 4 matmuls.
                lhsA = kT[h][0:64, p, :]
                lhsB = kT[h][64:128, p, :]
                for qg in range(0, nqb, 2):
                    qbs = [qb for qb in (qg, qg + 1) if qb < nqb]
                    scs = {}
                    for qb in qbs:
                        scs[qb] = ps.tile([P, 2, QB], f32, name=f"sc_{h}_{p}_{qb}", tag="sc")
                    for qb in qbs:
                        nc.tensor.matmul(
                            scs[qb][:, 0, :], lhsA, qT[h][0:64, qb * QB : (qb + 1) * QB],
                            start=True, stop=True,
                        )
                    for qb in qbs:
                        nc.tensor.matmul(
                            scs[qb][:, 1, :], lhsB, qT[h][64:128, qb * QB : (qb + 1) * QB],
                            start=True, stop=True,
                        )
                    for qb in qbs:
                        pT = ptp.tile([P, 2, QB], f32, name=f"pT_{h}_{p}_{qb}", tag="pT")
                        nc.scalar.activation(out=pT, in_=scs[qb], func=Exp, scale=SCALE)
                        pT_store[qb] = pT

            def emit_av(h, p, pT_prev, out_tiles):
                for half in range(2):
                    v_tile = vt[h][2 * p + half]
                    for qb in range(nqb):
                        nc.tensor.matmul(
                            out_tiles[qb][:, :],
                            v_tile,
                            pT_prev[qb][:, half, :],
                            start=(p == 0 and half == 0),
                            stop=(p == npair - 1 and half == 1),
                        )

            # ---- main schedule ----
            fillers = []

            def drain(n):
                for _ in range(min(n, len(fillers))):
                    fillers.pop(0)()

            def drain_all():
                drain(len(fillers))

            for op in input_stage(0):
                op()

            prev_out = None  # (head, out_tiles) pending output stage
            for h in range(hpc):
                out_tiles = [
                    acc.tile([D + 1, QB], f32, name=f"out_{h}_{qb}", tag="out")
                    for qb in range(nqb)
                ]
                if prev_out is not None:
                    fillers.extend(output_stage(*prev_out))
                if h + 1 < hpc:
                    fillers.extend(input_stage(h + 1))

                per_pair = -(-len(fillers) // npair) if fillers else 0
                pT_prev = None
                for p in range(npair):
                    pT_cur = {}
                    emit_qk_exp(h, p, pT_cur)
                    if pT_prev is not None:
                        emit_av(h, p - 1, pT_prev, out_tiles)
                    pT_prev = pT_cur
                    drain(per_pair)
                emit_av(h, npair - 1, pT_prev, out_tiles)
                drain_all()
                prev_out = (h, out_tiles)

            for op in output_stage(*prev_out):
                op()

    nc.compile()
    return nc


def _get_program():
    if "nc" not in _PROGRAM_CACHE:
        _PROGRAM_CACHE["nc"] = _build_program()
    return _PROGRAM_CACHE["nc"]


def kernel(query, key, value):
    from concourse import bass_utils

    nc = _get_program()
    q = np.ascontiguousarray(np.asarray(query, dtype=np.float32)).reshape(B * H, S, D)
    k = np.ascontiguousarray(np.asarray(key, dtype=np.float32)).reshape(B * H, S, D)
    v = np.ascontiguousarray(np.asarray(value, dtype=np.float32)).reshape(B * H, S, D)

    in_maps = [
        {
            "q": np.ascontiguousarray(q[c * HPC : (c + 1) * HPC]),
            "k": np.ascontiguousarray(k[c * HPC : (c + 1) * HPC]),
            "v": np.ascontiguousarray(v[c * HPC : (c + 1) * HPC]),
        }
        for c in range(N_CORES)
    ]
    res = bass_utils.run_bass_kernel_spmd(nc, in_maps, core_ids=list(range(N_CORES)))
    out = np.stack([res.results[c]["o"] for c in range(N_CORES)])  # [8, HPC, S, D]
    return out.reshape(B, H, S, D).astype(np.float32)


# revision 4
# speedup vs baseline: 1.5579x; 1.5579x over previous
"""Trainium2 Bass kernel: batched multi-head attention (B=2, H=16, S=2048, D=64).

out = 2.0 * softmax(0.2 * Q @ K^T) @ V, computed per (b, h) pair.

Sharding: the 32 (b,h) pairs are split across 8 NeuronCores, 4 heads per core.
Each core runs an identical Bass program on its own slice; no communication.

Per-head dataflow on a core (scores kept TRANSPOSED so exp is the only
elementwise pass over the S x S matrix):
  - Paired PE transposes build K^T with k-tile pairs stacked on partitions
    0-63 / 64-127, and Q^T duplicated on both partition halves.
  - QK^T: two concurrent K=64 matmuls on disjoint row groups write
    scores^T [128k x 2x512q] tiles into PSUM.
  - ACT exp(0.2*x) reads PSUM, writes P^T to SBUF (layout already correct
    for the AV matmul).
  - AV: lhsT = [V | 0.5] (65 cols), accumulating out^T[65, 512] over all 16
    k-tiles; row 64 = 0.5 * sum(exp) so its reciprocal is 2/denominator.
  - PE transposes out^T back in 128-col chunks; DVE applies the reciprocal;
    DMA out. Max subtraction is skipped: |0.2*scores| <= ~10, safe in fp32.
"""

import numpy as np

B, H, S, D = 2, 16, 2048, 64
N_CORES = 8
HPC = (B * H) // N_CORES  # heads per core = 4
SCALE = 0.2
FACTOR = 2.0

P = 128  # s-tile (partition) size
QB = 512  # q block = one PSUM bank of fp32

_PROGRAM_CACHE = {}


def _build_program(hpc=HPC, s=S):
    from contextlib import ExitStack

    import concourse.bass as bass  # noqa: F401
    import concourse.mybir as mybir
    import concourse.tile as tile
    from concourse import bacc
    from concourse.masks import make_identity

    f32 = mybir.dt.float32
    f32r = mybir.dt.float32r  # single-pass PE dtype: 4x faster than fp32, ~1e-4 rel err
    Exp = mybir.ActivationFunctionType.Exp

    nt = s // P  # s-tiles per head
    npair = nt // 2  # k-tile pairs
    nqb = s // QB  # q blocks per head

    nc = bacc.Bacc("TRN2", target_bir_lowering=False, debug=False)

    q_d = nc.dram_tensor("q", [hpc, s, D], f32, kind="ExternalInput").ap()
    k_d = nc.dram_tensor("k", [hpc, s, D], f32, kind="ExternalInput").ap()
    v_d = nc.dram_tensor("v", [hpc, s, D], f32, kind="ExternalInput").ap()
    o_d = nc.dram_tensor("o", [hpc, s, D], f32, kind="ExternalOutput").ap()

    with tile.TileContext(nc) as tc:
        with ExitStack() as ctx:
            const = ctx.enter_context(tc.tile_pool(name="const", bufs=1))
            pin = ctx.enter_context(tc.tile_pool(name="pin", bufs=6))
            kq = ctx.enter_context(tc.tile_pool(name="kq", bufs=2))
            vpool = ctx.enter_context(tc.tile_pool(name="vpool", bufs=2 * nt))
            ptp = ctx.enter_context(tc.tile_pool(name="ptp", bufs=8))
            otp = ctx.enter_context(tc.tile_pool(name="otp", bufs=3))
            osb = ctx.enter_context(tc.tile_pool(name="osb", bufs=6))
            # PSUM: "ps" = 2 slots x 2 banks (scores + transpose staging),
            # "acc" = 4 slots x 1 bank (out^T accumulators).
            ps = ctx.enter_context(tc.tile_pool(name="ps", bufs=2, space="PSUM"))
            acc = ctx.enter_context(tc.tile_pool(name="acc", bufs=4, space="PSUM"))

            ident = const.tile([P, P], f32)
            make_identity(nc, ident)

            # Warm the exp table set early so the ~2.7us load overlaps DMAs.
            warm = const.tile([P, 8], f32)
            nc.vector.memset(warm, 0.0)
            nc.scalar.activation(out=warm, in_=warm, func=Exp, scale=1.0)

            # Per-head persistent state (keyed by head index).
            kT = {}  # [128, npair, 128]; rows 0:63 = K_{2p}^T, 64:128 = K_{2p+1}^T
            qT = {}  # [128, s]; rows 0:63 = Q^T, rows 64:128 = duplicate
            vt = {}  # list of [128, 65] tiles: [V_t | 0.5]

            def load_k_pair(h, p):
                def op():
                    t = pin.tile([P, P], f32, name=f"kin_{h}_{p}", tag="pin")
                    nc.sync.dma_start(out=t[:, 0:D], in_=k_d[h, (2 * p) * P : (2 * p + 1) * P, :])
                    nc.sync.dma_start(out=t[:, D : 2 * D], in_=k_d[h, (2 * p + 1) * P : (2 * p + 2) * P, :])
                    tp = ps.tile([P, P], f32, name=f"ktp_{h}_{p}", tag="sc")
                    nc.tensor.transpose(tp, t, ident)
                    nc.vector.tensor_copy(out=kT[h][:, p, :], in_=tp)

                return op

            def load_q_tile(h, t_idx):
                def op():
                    t = pin.tile([P, P], f32, name=f"qin_{h}_{t_idx}", tag="pin")
                    src = q_d[h, t_idx * P : (t_idx + 1) * P, :]
                    nc.sync.dma_start(out=t[:, 0:D], in_=src)
                    nc.sync.dma_start(out=t[:, D : 2 * D], in_=src)
                    tp = ps.tile([P, P], f32, name=f"qtp_{h}_{t_idx}", tag="sc")
                    nc.tensor.transpose(tp, t, ident)
                    nc.vector.tensor_copy(out=qT[h][:, t_idx * P : (t_idx + 1) * P], in_=tp)

                return op

            def load_v_tile(h, t_idx):
                def op():
                    st = pin.tile([P, D + 1], f32, name=f"vst_{h}_{t_idx}", tag="vstage")
                    nc.sync.dma_start(out=st[:, 0:D], in_=v_d[h, t_idx * P : (t_idx + 1) * P, :])
                    nc.vector.memset(st[:, D : D + 1], 0.5)
                    t = vpool.tile([P, D + 1], f32r, name=f"v_{h}_{t_idx}", tag="v")
                    nc.vector.tensor_copy(out=t, in_=st)
                    vt[h].append(t)

                return op

            def input_stage(h):
                kT[h] = kq.tile([P, npair, P], f32r, name=f"kT_{h}", tag="kT")
                qT[h] = kq.tile([P, s], f32r, name=f"qT_{h}", tag="qT")
                vt[h] = []
                ops = []
                for p in range(npair):
                    ops.append(load_k_pair(h, p))
                for t_idx in range(nt):
                    ops.append(load_q_tile(h, t_idx))
                for t_idx in range(nt):
                    ops.append(load_v_tile(h, t_idx))
                return ops

            def out_chunk(h, qb, oT, c):
                def op():
                    tp = ps.tile([P, D + 1], f32, name=f"otr_{h}_{qb}_{c}", tag="sc")
                    nc.tensor.transpose(tp, oT[:, c * P : (c + 1) * P], ident[0 : D + 1, 0 : D + 1])
                    recip = osb.tile([P, 1], f32, name=f"rcp_{h}_{qb}_{c}", tag="rcp")
                    nc.vector.reciprocal(out=recip, in_=tp[:, D : D + 1])
                    ob = osb.tile([P, D], f32, name=f"ob_{h}_{qb}_{c}", tag="ob")
                    nc.vector.tensor_scalar_mul(out=ob, in0=tp[:, 0:D], scalar1=recip)
                    nc.sync.dma_start(out=o_d[h, qb * QB + c * P : qb * QB + (c + 1) * P, :], in_=ob)

                return op

            def output_stage(h, out_tiles):
                ops = []
                oTs = {}

                def evac(qb):
                    def op():
                        oT = otp.tile([D + 1, QB], f32, name=f"oT_{h}_{qb}", tag="oT")
                        nc.vector.tensor_copy(out=oT, in_=out_tiles[qb])
                        oTs[qb] = oT

                    return op

                for qb in range(nqb):
                    ops.append(evac(qb))
                for qb in range(nqb):
                    for c in range(QB // P):
                        # oTs[qb] resolved lazily (evac runs first in emission order)
                        ops.append(lambda qb=qb, c=c: out_chunk(h, qb, oTs[qb], c)())
                return ops

            def emit_qk_exp(h, p, pT_store):
                # QK^T + exp for pair p over all q-blocks, two q-blocks at a
                # time so the 2 score slots never deadlock and LDWEIGHTS
                # switches only 2x per group of 4 matmuls.
                lhsA = kT[h][0:64, p, :]
                lhsB = kT[h][64:128, p, :]
                for qg in range(0, nqb, 2):
                    qbs = [qb for qb in (qg, qg + 1) if qb < nqb]
                    scs = {}
                    for qb in qbs:
                        scs[qb] = ps.tile([P, 2, QB], f32, name=f"sc_{h}_{p}_{qb}", tag="sc")
                    for qb in qbs:
                        nc.tensor.matmul(
                            scs[qb][:, 0, :], lhsA, qT[h][0:64, qb * QB : (qb + 1) * QB],
                            start=True, stop=True,
                        )
                    for qb in qbs:
                        nc.tensor.matmul(
                            scs[qb][:, 1, :], lhsB, qT[h][64:128, qb * QB : (qb + 1) * QB],
                            start=True, stop=True,
                        )
                    for qb in qbs:
                        pT = ptp.tile([P, 2, QB], f32r, name=f"pT_{h}_{p}_{qb}", tag="pT")
                        nc.scalar.activation(out=pT, in_=scs[qb], func=Exp, scale=SCALE)
                        pT_store[qb] = pT

            def emit_av(h, p, pT_prev, out_tiles):
                for half in range(2):
                    v_tile = vt[h][2 * p + half]
                    for qb in range(nqb):
                        nc.tensor.matmul(
                            out_tiles[qb][:, :],
                            v_tile,
                            pT_prev[qb][:, half, :],
                            start=(p == 0 and half == 0),
                            stop=(p == npair - 1 and half == 1),
                        )

            # ---- main schedule ----
            fillers = []

            def drain(n):
                for _ in range(min(n, len(fillers))):
                    fillers.pop(0)()

            def drain_all():
                drain(len(fillers))

            for op in input_stage(0):
                op()

            prev_out = None  # (head, out_tiles) pending output stage
            for h in range(hpc):
                out_tiles = [
                    acc.tile([D + 1, QB], f32, name=f"out_{h}_{qb}", tag="out")
                    for qb in range(nqb)
                ]
                if prev_out is not None:
                    fillers.extend(output_stage(*prev_out))
                if h + 1 < hpc:
                    fillers.extend(input_stage(h + 1))

                per_pair = -(-len(fillers) // npair) if fillers else 0
                pT_prev = None
                for p in range(npair):
                    pT_cur = {}
                    emit_qk_exp(h, p, pT_cur)
                    if pT_prev is not None:
                        emit_av(h, p - 1, pT_prev, out_tiles)
                    pT_prev = pT_cur
                    drain(per_pair)
                emit_av(h, npair - 1, pT_prev, out_tiles)
                drain_all()
                prev_out = (h, out_tiles)

            for op in output_stage(*prev_out):
                op()

    nc.compile()
    return nc


def _get_program():
    if "nc" not in _PROGRAM_CACHE:
        _PROGRAM_CACHE["nc"] = _build_program()
    return _PROGRAM_CACHE["nc"]


def kernel(query, key, value):
    from concourse import bass_utils

    nc = _get_program()
    q = np.ascontiguousarray(np.asarray(query, dtype=np.float32)).reshape(B * H, S, D)
    k = np.ascontiguousarray(np.asarray(key, dtype=np.float32)).reshape(B * H, S, D)
    v = np.ascontiguousarray(np.asarray(value, dtype=np.float32)).reshape(B * H, S, D)

    in_maps = [
        {
            "q": np.ascontiguousarray(q[c * HPC : (c + 1) * HPC]),
            "k": np.ascontiguousarray(k[c * HPC : (c + 1) * HPC]),
            "v": np.ascontiguousarray(v[c * HPC : (c + 1) * HPC]),
        }
        for c in range(N_CORES)
    ]
    res = bass_utils.run_bass_kernel_spmd(nc, in_maps, core_ids=list(range(N_CORES)))
    out = np.stack([res.results[c]["o"] for c in range(N_CORES)])  # [8, HPC, S, D]
    return out.reshape(B, H, S, D).astype(np.float32)
